# revision 27
# baseline (speedup 1.0000x reference)
"""GATv2 (3-layer, 4-head) message-passing kernel for Trainium2, 8-core SPMD.

V5 design: degree-bucketed destination-aligned edge layout.

Nodes are sorted by (in-)degree and dealt round-robin to the 8 cores, so each
core's chunk t of 128 destination slots covers the global degree-rank band
[1024t, 1024t+1024) and every core shares one per-chunk edge depth J[t]
(SPMD: one program).  Edge slot (p, j) of a chunk holds the j-th incoming
edge of destination slot p — so xr[dst] is a per-partition broadcast, the
segment softmax denominator and the weighted feature sum are per-partition
reductions over j (done on the PE via identity-matmul accumulation), and the
V4 selection-matrix machinery (is_equal S build + dstl) disappears entirely.

Gathers (int16 SWDGE indices cap tables at 32768 rows):
  The AllGather table has 8*(6250+1) = 50008 rows; each core appends one
  ZERO row after its shard, giving known-zero rows in both halves.
  Per chunk, hi-table edges are placed FIRST (j < nhi_p), so
    gather_hi: table rows [32768, 50008), covers j < Jhi[t]  (pads -> zero row)
    gather_lo: table rows [0, 32768),     covers j < J[t]    (pads -> zero row)
  and m = (g_lo[:, :Jhi] += g_hi) + xr broadcast needs no per-slot indexing.

Per-edge math (the V4 algebraic trick, per-partition denominators):
  m = xl[src] + xr[dst];  lk = prelu(m, 0.2) * attJ;  e = sum_c lk
  ee = exp(e) * mask;  po = sum_j [m*ee | ee]  (PE identity accumulation)
  out = po[:,0:D]/po[:,D:] - xr + (bl + bias); then residual + LayerNorm.

ACT stays inside ONE function table set (natural_log_exp_and_others:
copy/prelu/exp/ln/relu), with rstd = exp(-0.5*ln(var+eps)) — no per-chunk
ACT_TABLE_LOAD switches (V4 paid 2x ~2.6us per chunk for Exp<->Sqrt).
"""

import os
import sys

sys.path.insert(0, "/opt/trn_rl_repo")

import ml_dtypes
import numpy as np

import concourse.bass as bass
import concourse.bacc as bacc
import concourse.tile as tile
from concourse import mybir

F32 = mybir.dt.float32
I32 = mybir.dt.int32
I16 = mybir.dt.int16
BF16 = mybir.dt.bfloat16
AF = mybir.ActivationFunctionType
ALU = mybir.AluOpType
AX = mybir.AxisListType

P = 128
NEG_SLOPE = 0.2
LN_EPS = 1e-5
DENOM_EPS = 1e-30
TBL_SPLIT = 32768

DBG_LAYERS = int(os.environ.get("GAT_LAYERS", "0"))  # 0 = all


class Cfg:
    def __init__(self, N=50000, D=128, H=4, L=3, n_cores=8):
        self.N, self.D, self.H, self.L, self.M = N, D, H, L, n_cores
        self.C = D // H
        assert N % n_cores == 0
        self.shard = N // n_cores  # 6250
        self.chunks = (self.shard + P - 1) // P  # 49
        self.tshard = self.shard + 1  # shard + zero row
        self.trows = self.tshard * n_cores  # 50008


# ----------------------------------------------------------------------------
# Host preprocessing
# ----------------------------------------------------------------------------

def _wrap16(a):
    """Linear idx list [n] -> dma_gather layout [128, n/16] int16 (value for
    gathered row i sits at partition i%16, col i//16; replicated across the 8
    Q7 groups)."""
    n = len(a)
    assert n % 16 == 0
    a16 = a.reshape(-1, 16).T.astype(np.int16)
    return np.ascontiguousarray(np.tile(a16, (8, 1)))


def preprocess(edge_index, cfg):
    N, M, shard, chunks = cfg.N, cfg.M, cfg.shard, cfg.chunks
    ei = np.asarray(edge_index)
    loops = np.arange(N, dtype=np.int64)
    src = np.concatenate([ei[0].astype(np.int64), loops])
    dst = np.concatenate([ei[1].astype(np.int64), loops])

    deg = np.bincount(dst, minlength=N)

    # One fixpoint step: a degree-only sort gives provisional table rows and
    # hence provisional per-dst lo/hi counts, which become the secondary sort
    # key; the FINAL rows (and true nlo/nhi used for placement) come from the
    # final (deg desc, nlo desc) order. Chunks then have tight nlo AND nhi
    # ranges -> compact disjoint lo/hi gather regions.
    order0 = np.argsort(-deg, kind="stable")
    rank0 = np.empty(N, np.int64)
    rank0[order0] = np.arange(N)
    row0 = (rank0 % M) * cfg.tshard + rank0 // M
    hi0 = row0[src] >= TBL_SPLIT
    nlo0 = deg - np.bincount(
        dst, weights=hi0.astype(np.float64), minlength=N
    ).astype(np.int64)

    order = np.lexsort((-nlo0, -deg))  # rank -> node
    rank_of = np.empty(N, np.int64)
    rank_of[order] = np.arange(N)
    core_of = rank_of % M
    slot_of = rank_of // M
    table_row = core_of * cfg.tshard + slot_of  # FINAL AllGather-table rows
    ishi_e = table_row[src] >= TBL_SPLIT
    nhi = np.bincount(dst, weights=ishi_e.astype(np.float64), minlength=N)
    nhi = nhi.astype(np.int64)
    nlo = deg - nhi

    # zero rows: core c's row c*tshard + shard
    ZLO = 0 * cfg.tshard + shard  # 6250 (< 32768)
    ZHI = (M - 1) * cfg.tshard + shard - TBL_SPLIT  # 50007 - 32768 = 17239
    assert ZLO < TBL_SPLIT and 0 <= ZHI < 32768

    # per-chunk shared depths from global rank bands
    band = P * M
    Jlo = np.zeros(chunks, np.int64)
    Jhi = np.zeros(chunks, np.int64)
    for t in range(chunks):
        b = order[t * band : min(t * band + band, N)]
        Jlo[t] = nlo[b].max()
        Jhi[t] = nhi[b].max()
    Jlo = np.maximum(Jlo, 1)
    Jhi = np.maximum(Jhi, 1)
    J = Jlo + Jhi

    # per-edge placement: lo edges at j in [0, nlo_p), hi at [Jlo, Jlo+nhi_p)
    core_e = core_of[dst]
    slot_e = slot_of[dst]
    srow_e = table_row[src]
    oe = np.lexsort((np.arange(len(src)), ishi_e, slot_e, core_e))
    core_s, slot_s, srow_s, ishi_s = core_e[oe], slot_e[oe], srow_e[oe], ishi_e[oe]
    gid = core_s * shard + slot_s
    newgrp = np.r_[True, gid[1:] != gid[:-1]]
    idx = np.arange(len(gid))
    grp_start = np.maximum.accumulate(np.where(newgrp, idx, 0))
    jpos = idx - grp_start  # j within dst (lo edges first, 0-based)
    # hi edges' local index within the hi block: jpos - nlo[dst]
    dst_node_s = dst[oe]
    jhi_pos = jpos - nlo[dst_node_s]

    ch_s = slot_s // P
    p_s = slot_s % P

    pre = []
    for c in range(M):
        selc = core_s == c
        idx_lo, idx_hi = [], []
        for t in range(chunks):
            Jlt, Jht = int(Jlo[t]), int(Jhi[t])
            sel = selc & (ch_s == t)
            sj, sp, sr, sh = jpos[sel], p_s[sel], srow_s[sel], ishi_s[sel]
            sjh = jhi_pos[sel]
            A = np.full(Jlt * P, ZLO, dtype=np.int64)
            lo = ~sh
            A[sj[lo] * P + sp[lo]] = sr[lo]
            B = np.full(Jht * P, ZHI, dtype=np.int64)
            B[sjh[sh] * P + sp[sh]] = sr[sh] - TBL_SPLIT
            assert A.max() < TBL_SPLIT and B.max() < TBL_SPLIT
            idx_lo.append(_wrap16(A))
            idx_hi.append(_wrap16(B))
        pre.append({"idx_lo": idx_lo, "idx_hi": idx_hi})

    meta = {
        "J": tuple(int(x) for x in J),
        "Jlo": tuple(int(x) for x in Jlo),
        "Jhi": tuple(int(x) for x in Jhi),
        "order": order,
        "core_of": core_of,
        "slot_of": slot_of,
    }
    return pre, meta


# ----------------------------------------------------------------------------
# Kernel builder
# ----------------------------------------------------------------------------

def build(tc, io, cfg, meta):
    from contextlib import ExitStack

    nc = tc.nc
    D, H, L, C = cfg.D, cfg.H, cfg.L, cfg.C
    shard, chunks = cfg.shard, cfg.chunks
    J, Jhi = meta["J"], meta["Jhi"]
    Jmax = max(J)

    Jlo = meta["Jlo"]
    olo = np.concatenate([[0], np.cumsum(np.array(Jlo) * 8)])
    ohi = np.concatenate([[0], np.cumsum(np.array(Jhi) * 8)])

    ctx = ExitStack()
    dram = ctx.enter_context(tc.tile_pool(name="drampool", bufs=1, space="DRAM"))
    consts = ctx.enter_context(tc.tile_pool(name="consts", bufs=1))
    lconsts = ctx.enter_context(tc.tile_pool(name="lconsts", bufs=2))
    xtp = ctx.enter_context(tc.tile_pool(name="xtp", bufs=1))
    nodep = ctx.enter_context(tc.tile_pool(name="nodep", bufs=3))
    edgep = ctx.enter_context(tc.tile_pool(name="edgep", bufs=3))
    smallp = ctx.enter_context(tc.tile_pool(name="smallp", bufs=4))
    ps_o = ctx.enter_context(tc.tile_pool(name="ps_o", bufs=2, space="PSUM"))
    ps_n = ctx.enter_context(tc.tile_pool(name="ps_n", bufs=2, space="PSUM"))
    ps_t = ctx.enter_context(tc.tile_pool(name="ps_t", bufs=2, space="PSUM"))

    # internal DRAM
    xl_sh = [dram.tile([cfg.tshard, D], BF16, name=f"xl_sh{l}") for l in range(L)]
    xl_all = [
        dram.tile([cfg.trows, D], BF16, name=f"xl_all{l}", addr_space="Shared")
        for l in range(L)
    ]

    # constants resident in SBUF
    ident_sb = consts.tile([P, P], F32, name="ident_sb")
    nc.sync.dma_start(out=ident_sb[:], in_=io["ident"][:, :])
    identb_sb = consts.tile([P, P], BF16, name="identb_sb")
    nc.sync.dma_start(out=identb_sb[:], in_=io["identb"][:, :])
    zrow_sb = consts.tile([P, D], BF16, name="zrow_sb")
    nc.vector.memset(zrow_sb[:], 0.0)
    eps_sb = consts.tile([P, 1], F32, name="eps_sb")
    nc.vector.memset(eps_sb[:], LN_EPS)

    # Pin the ACT table to natural_log_exp_and_others (set 6: copy, prelu,
    # exp, ln, relu) — one load for the whole program, no per-chunk switches.
    nc.scalar.add_instruction(
        mybir.InstLoadActFuncSet(act_func_set_id=6, name="act_set6_load")
    )

    # layer-invariant per-chunk tables, SBUF-resident
    idxlo_sb = consts.tile([P, int(olo[-1])], I16, name="idxlo_sb")
    nc.sync.dma_start(out=idxlo_sb[:], in_=io["idxlo_all"][:, :])
    idxhi_sb = consts.tile([P, int(ohi[-1])], I16, name="idxhi_sb")
    nc.sync.dma_start(out=idxhi_sb[:], in_=io["idxhi_all"][:, :])

    # x transposed [D, shard] + residual input [node, D], SBUF-resident.
    # Ping-pong by layer so cross-chunk subtile writes never chain onto the
    # previous layer's reads (whole-tile dependency serialization).
    xT2 = [xtp.tile([P, chunks * P], BF16, name=f"xT_sb{i}") for i in range(2)]
    xq2 = [xtp.tile([P, chunks * D], BF16, name=f"xq_sb{i}") for i in range(2)]
    xr_sb = xtp.tile([P, chunks * D], BF16, name="xr_sb")

    # prologue: transpose x_shard into xT2[0]; keep x in xq2[0]
    for t in range(chunks):
        nt = min(P, shard - t * P)
        xq0 = nodep.tile([P, D], F32, name="xq0")
        nc.sync.dma_start(out=xq0[:nt, :], in_=io["x_shard"][t * P : t * P + nt, :])
        nc.scalar.activation(
            out=xq2[0][:nt, t * D : (t + 1) * D], in_=xq0[:nt, :], func=AF.Copy
        )
        psT = ps_t.tile([P, 512], F32, name="psT", tag="psT")
        nc.tensor.transpose(
            out=psT[:, :nt], in_=xq0[:nt, :], identity=ident_sb[:nt, :nt]
        )
        nc.scalar.activation(
            out=xT2[0][:, t * P : t * P + nt], in_=psT[:, :nt], func=AF.Copy
        )

    L_eff = DBG_LAYERS if DBG_LAYERS else L
    for l in range(L_eff):
        xT_sb = xT2[l % 2]
        xq_sb = xq2[l % 2]
        xT_nx = xT2[(l + 1) % 2]
        xq_nx = xq2[(l + 1) % 2]
        # per-layer constants
        wl_sb = lconsts.tile([P, D], BF16, name="wl_sb")
        nc.sync.dma_start(out=wl_sb[:], in_=io["Wl16"][l, :, :])
        wr_sb = lconsts.tile([P, D], BF16, name="wr_sb")
        nc.sync.dma_start(out=wr_sb[:], in_=io["Wr16"][l, :, :])
        attJ_sb = lconsts.tile([P, Jmax * D], BF16, name="attJ_sb")
        nc.gpsimd.dma_start(
            out=attJ_sb[:], in_=_row_bcast(io["attJ16"], l, P, Jmax * D)
        )
        bc_sb = lconsts.tile([P, D], F32, name="bc_sb")
        nc.gpsimd.dma_start(out=bc_sb[:], in_=_row_bcast(io["bc"], l, P, D))
        cvec_sb = lconsts.tile([P, D], F32, name="cvec_sb")
        nc.gpsimd.dma_start(out=cvec_sb[:], in_=_row_bcast(io["cvec"], l, P, D))
        gamma_sb = lconsts.tile([P, D], F32, name="gamma_sb")
        nc.gpsimd.dma_start(out=gamma_sb[:], in_=_row_bcast(io["gamma"], l, P, D))
        beta_sb = lconsts.tile([P, D], F32, name="beta_sb")
        nc.gpsimd.dma_start(out=beta_sb[:], in_=_row_bcast(io["beta"], l, P, D))

        # --------------------------------------------------------------
        # node phase: xl = x@Wl -> xl_sh (bf16); xr = x@Wr + (bl+br) -> xr_sb
        # --------------------------------------------------------------
        for t in range(chunks):
            nt = min(P, shard - t * P)
            lhsT = xT_sb[:, t * P : t * P + nt]
            ps_xl = ps_n.tile([P, 512], F32, name="ps_xl", tag="ps_n")
            nc.tensor.matmul(
                out=ps_xl[:nt, 0:D], lhsT=lhsT, rhs=wl_sb[:], start=True, stop=True
            )
            xl_o = nodep.tile([P, D], BF16, name="xl_o")
            nc.scalar.activation(out=xl_o[:nt, :], in_=ps_xl[:nt, 0:D], func=AF.Copy)
            nc.sync.dma_start(out=xl_sh[l][t * P : t * P + nt, :], in_=xl_o[:nt, :])

            ps_xr = ps_n.tile([P, 512], F32, name="ps_xr", tag="ps_n")
            nc.tensor.matmul(
                out=ps_xr[:nt, 0:D], lhsT=lhsT, rhs=wr_sb[:], start=True, stop=True
            )
            nc.vector.tensor_tensor(
                out=xr_sb[:nt, t * D : (t + 1) * D], in0=ps_xr[:nt, 0:D],
                in1=bc_sb[:nt, :], op=ALU.add,
            )
        # pad row = -1e4*sign(att): pad-slot logits underflow exp() to 0,
        # so no per-edge validity mask is needed.
        pad_sb = nodep.tile([P, D], BF16, name="pad_sb")
        nc.sync.dma_start(out=pad_sb[0:1, :], in_=io["sgn16"][l, :, :])
        nc.sync.dma_start(out=xl_sh[l][shard : shard + 1, :], in_=pad_sb[0:1, :])

        # --------------------------------------------------------------
        # AllGather xl across the 8 cores
        # --------------------------------------------------------------
        nc.gpsimd.collective_compute(
            "AllGather",
            ALU.bypass,
            replica_groups=[list(range(cfg.M))],
            ins=[xl_sh[l][:, :].opt()],
            outs=[xl_all[l][:, :].opt()],
        )

        # --------------------------------------------------------------
        # edge phase
        # --------------------------------------------------------------
        for ch in range(chunks):
            Jt, Jlt, Jht = J[ch], Jlo[ch], Jhi[ch]
            nt = min(P, shard - ch * P)
            rows = slice(ch * P, ch * P + nt)
            xr_ch = xr_sb[:, ch * D : (ch + 1) * D]

            # four gathers (one per SWDGE queue / Q7 core pair, concurrent)
            # fill disjoint j-ranges of one tile; pads hit shard-end zero rows
            g = edgep.tile([P, Jt, D], BF16, name="g")
            la = (Jlt + 1) // 2
            ha = (Jht + 1) // 2
            parts = [
                (0, la, idxlo_sb, olo[ch] * 1, 0, 0),
                (la, Jlt, idxlo_sb, olo[ch] + la * 8, 0, 1),
                (Jlt, Jlt + ha, idxhi_sb, ohi[ch] * 1, 1, 2),
                (Jlt + ha, Jt, idxhi_sb, ohi[ch] + ha * 8, 1, 3),
            ]
            for j0, j1, itile, ioff, ishi, q in parts:
                nj = j1 - j0
                if nj <= 0:
                    continue
                tbl = (
                    xl_all[l][TBL_SPLIT : cfg.trows, :]
                    if ishi
                    else xl_all[l][0:TBL_SPLIT, :]
                )
                nc.gpsimd.dma_gather(
                    out_ap=g[:, j0:j1, :],
                    in_ap=tbl,
                    idxs_ap=itile[:, int(ioff) : int(ioff) + nj * 8],
                    num_idxs=nj * P,
                    num_idxs_reg=nj * P,
                    elem_size=D,
                    single_packet=False,
                    queue_num=q,
                )

            # m = xl[src] + xr[dst]  (in place: g becomes m)
            nc.vector.tensor_tensor(
                out=g[:, :, :],
                in0=g[:, :, :],
                in1=xr_ch.unsqueeze(1).to_broadcast([P, Jt, D]),
                op=ALU.add,
            )

            # lk = prelu(m) * att (att pre-replicated along J)
            lk = edgep.tile([P, Jt, D], BF16, name="lk")
            nc.scalar.activation(
                out=lk[:, :, :], in_=g[:, :, :], func=AF.Prelu, alpha=NEG_SLOPE
            )
            nc.vector.tensor_tensor(
                out=lk[:, :, :],
                in0=lk[:, :, :],
                in1=attJ_sb[:, 0 : Jt * D].rearrange("p (j d) -> p j d", j=Jt),
                op=ALU.mult,
            )
            lg = smallp.tile([P, Jt, H], F32, name="lg")
            nc.vector.reduce_sum(
                out=lg[:, :, :],
                in_=lk[:, :, :].rearrange("p j (h c) -> p j h c", h=H),
                axis=AX.X,
            )

            # ee = exp(e) * mask (small); denominators on the DVE (tiny
            # strided reduce); ee broadcast over head cols via ACT into lk
            # (dead after lg), then lk *= m  ->  lk = ee * m
            eem = smallp.tile([P, Jt, H], BF16, name="eem")
            nc.scalar.activation(out=eem[:, :, :], in_=lg[:, :, :], func=AF.Exp)
            dns = smallp.tile([P, H], F32, name="dns")
            nc.vector.reduce_sum(
                out=dns[:, :],
                in_=eem[:, :, :].rearrange("p j h -> p h j"),
                axis=AX.X,
            )
            nc.scalar.activation(
                out=lk[:, :, :].rearrange("p j (h c) -> p j h c", h=H),
                in_=eem[:, :, :].unsqueeze(3).to_broadcast([P, Jt, H, C]),
                func=AF.Copy,
            )
            nc.vector.tensor_tensor(
                out=lk[:, :, :], in0=lk[:, :, :], in1=g[:, :, :], op=ALU.mult
            )

            # pairwise pre-add on DVE halves the PE matmul convoy
            J2 = Jt // 2
            if J2 > 0:
                v = lk[:, 0 : 2 * J2, :].rearrange(
                    "p (j two) d -> p j two d", two=2
                )
                nc.vector.tensor_tensor(
                    out=lk[:, 0:J2, :], in0=v[:, :, 0, :], in1=v[:, :, 1, :],
                    op=ALU.add,
                )
            # per-dst weighted sums over j on the PE: po = sum_j ee*m
            po_b = ps_o.tile([P, 512], F32, name="po")
            po = po_b[:, 0:D]
            rhs_js = list(range(J2)) + ([Jt - 1] if Jt % 2 else [])
            for k, j in enumerate(rhs_js):
                nc.tensor.matmul(
                    out=po[:, :],
                    lhsT=identb_sb[:, :],
                    rhs=lk[:, j, :],
                    start=(k == 0),
                    stop=(k == len(rhs_js) - 1),
                )

            dn = smallp.tile([P, H], F32, name="dn")
            nc.vector.tensor_scalar(
                out=dn[:, :], in0=dns[:, :], scalar1=DENOM_EPS,
                scalar2=None, op0=ALU.add,
            )
            rd = smallp.tile([P, H], F32, name="rd")
            nc.vector.reciprocal(out=rd[:, :], in_=dn[:, :])

            onrm = smallp.tile([P, D], F32, name="onrm")
            nc.vector.tensor_tensor(
                out=onrm[:, :].rearrange("p (h c) -> p h c", h=H),
                in0=po[:, :].rearrange("p (h c) -> p h c", h=H),
                in1=rd[:, :].unsqueeze(2).to_broadcast([P, H, C]),
                op=ALU.mult,
            )

            # h = onrm - xr[dst] + (bl + gat_bias); then residual + LN
            # (in-place chain on the onrm tile)
            t3 = onrm
            nc.vector.tensor_tensor(
                out=t3[:nt, :], in0=onrm[:nt, :], in1=xr_ch[:nt, :],
                op=ALU.subtract,
            )
            nc.vector.tensor_tensor(
                out=t3[:nt, :], in0=t3[:nt, :], in1=cvec_sb[:nt, :], op=ALU.add
            )
            nc.vector.tensor_tensor(
                out=t3[:nt, :], in0=t3[:nt, :],
                in1=xq_sb[:nt, ch * D : (ch + 1) * D], op=ALU.add,
            )

            st6 = smallp.tile([P, 6], F32, name="st6")
            nc.vector.bn_stats(out=st6[:nt, :], in_=t3[:nt, :])
            mv = smallp.tile([P, 2], F32, name="mv")
            nc.vector.bn_aggr(out=mv[:nt, :], in_=st6[:nt, :])
            # rstd = exp(-0.5 * ln(var + eps)) — both funcs live in set 6
            lnv = smallp.tile([P, 1], F32, name="lnv")
            nc.scalar.activation(
                out=lnv[:nt, :], in_=mv[:nt, 1:2], func=AF.Ln, bias=eps_sb[:nt, :]
            )
            rstd = smallp.tile([P, 1], F32, name="rstd")
            nc.scalar.activation(
                out=rstd[:nt, :], in_=lnv[:nt, :], func=AF.Exp, scale=-0.5
            )

            y3 = smallp.tile([P, D], F32, name="y3")
            nc.vector.tensor_scalar(
                out=y3[:nt, :], in0=t3[:nt, :], scalar1=mv[:nt, 0:1],
                scalar2=rstd[:nt, :], op0=ALU.subtract, op1=ALU.mult,
            )
            nc.vector.tensor_tensor(
                out=y3[:nt, :], in0=y3[:nt, :], in1=gamma_sb[:nt, :], op=ALU.mult
            )
            nc.vector.tensor_tensor(
                out=y3[:nt, :], in0=y3[:nt, :], in1=beta_sb[:nt, :], op=ALU.add
            )

            if l < L_eff - 1:
                # relu -> next layer's residual input (SBUF) + transpose
                nc.scalar.activation(
                    out=xq_nx[:nt, ch * D : (ch + 1) * D], in_=y3[:nt, :],
                    func=AF.Relu,
                )
                psT2 = ps_t.tile([P, 1024], BF16, name="psT2", tag="psT")
                nc.tensor.transpose(
                    out=psT2[:, :nt], in_=xq_nx[:nt, ch * D : (ch + 1) * D],
                    identity=identb_sb[:nt, :nt],
                )
                nc.scalar.activation(
                    out=xT_nx[:, ch * P : ch * P + nt], in_=psT2[:, :nt],
                    func=AF.Copy,
                )
            else:
                nc.sync.dma_start(out=io["y"][rows, :], in_=y3[:nt, :])

    ctx.close()


def _row_bcast(ap, row, parts, d):
    """AP reading row `row` of a [R, 1, D] or [R, D] DRAM tensor, replicated
    across `parts` partitions (partition step 0)."""
    flat = ap[row] if ap.ndim == 3 else ap[row : row + 1]
    base = flat.opt()
    return bass.AP(tensor=base.tensor, offset=row * d, ap=[[0, parts], [1, d]])


# ----------------------------------------------------------------------------
# host-side inputs
# ----------------------------------------------------------------------------

def make_host_inputs(inputs, cfg, meta):
    L, D, H, C = cfg.L, cfg.D, cfg.H, cfg.C
    Jmax = max(meta["J"])
    Wl = np.asarray(inputs["Wl"], np.float32)
    bl = np.asarray(inputs["bl"], np.float32)
    br = np.asarray(inputs["br"], np.float32)
    att = np.asarray(inputs["att"], np.float32)
    gat_bias = np.asarray(inputs["bias"], np.float32)
    gamma = np.asarray(inputs["gamma"], np.float32)
    beta = np.asarray(inputs["beta"], np.float32)
    attJ = np.tile(att.reshape(L, 1, H * C), (1, Jmax, 1)).reshape(L, 1, Jmax * D)
    return {
        "Wl16": Wl.astype(ml_dtypes.bfloat16),
        "Wr16": np.asarray(inputs["Wr"], np.float32).astype(ml_dtypes.bfloat16),
        "attJ16": attJ.astype(ml_dtypes.bfloat16),
        "bc": (bl + br).reshape(L, 1, D),
        "cvec": (bl + gat_bias).reshape(L, 1, D),
        "gamma": gamma.reshape(L, 1, D),
        "beta": beta.reshape(L, 1, D),
        "sgn16": (-1e4 * np.sign(att).reshape(L, 1, D)).astype(
            ml_dtypes.bfloat16
        ),
        "ident": np.eye(P, dtype=np.float32),
        "identb": np.eye(P, dtype=np.float32).astype(ml_dtypes.bfloat16),
    }


def make_in_maps(inputs, pre, cfg, meta):
    x = np.asarray(inputs["fine_poi_x"], np.float32)
    shared = make_host_inputs(inputs, cfg, meta)
    order, core_of, slot_of = meta["order"], meta["core_of"], meta["slot_of"]
    in_maps = []
    for c in range(cfg.M):
        m = dict(shared)
        # x rows of core c in slot order: node at (c, slot s) = order[s*M + c]
        nodes = order[np.arange(cfg.shard) * cfg.M + c]
        m["x_shard"] = np.ascontiguousarray(x[nodes])
        m["idxlo_all"] = np.concatenate(pre[c]["idx_lo"], axis=1)
        m["idxhi_all"] = np.concatenate(pre[c]["idx_hi"], axis=1)
        in_maps.append(m)
    return in_maps


# ----------------------------------------------------------------------------
# program assembly + execution
# ----------------------------------------------------------------------------

_CACHE = {}


def _build_program(cfg, meta):
    key = (cfg.N, cfg.D, cfg.H, cfg.L, cfg.M, meta["Jlo"], meta["Jhi"])
    if key in _CACHE:
        return _CACHE[key]
    nc = bacc.Bacc(
        "TRN2", target_bir_lowering=False, debug=False, num_devices=cfg.M,
        num_swdge_queues=4,
    )
    J, Jhi = meta["J"], meta["Jhi"]
    Jmax = max(J)
    io = {}
    io["x_shard"] = nc.dram_tensor(
        "x_shard", [cfg.shard, cfg.D], F32, kind="ExternalInput"
    ).ap()
    Jlo = meta["Jlo"]
    io["idxlo_all"] = nc.dram_tensor(
        "idxlo_all", [P, sum(Jlo) * 8], I16, kind="ExternalInput"
    ).ap()
    io["idxhi_all"] = nc.dram_tensor(
        "idxhi_all", [P, sum(Jhi) * 8], I16, kind="ExternalInput"
    ).ap()
    io["sgn16"] = nc.dram_tensor(
        "sgn16", [cfg.L, 1, cfg.D], BF16, kind="ExternalInput"
    ).ap()
    io["Wl16"] = nc.dram_tensor(
        "Wl16", [cfg.L, cfg.D, cfg.D], BF16, kind="ExternalInput"
    ).ap()
    io["Wr16"] = nc.dram_tensor(
        "Wr16", [cfg.L, cfg.D, cfg.D], BF16, kind="ExternalInput"
    ).ap()
    io["attJ16"] = nc.dram_tensor(
        "attJ16", [cfg.L, 1, Jmax * cfg.D], BF16, kind="ExternalInput"
    ).ap()
    for nm in ["bc", "cvec", "gamma", "beta"]:
        io[nm] = nc.dram_tensor(
            nm, [cfg.L, 1, cfg.D], F32, kind="ExternalInput"
        ).ap()
    io["ident"] = nc.dram_tensor("ident", [P, P], F32, kind="ExternalInput").ap()
    io["identb"] = nc.dram_tensor("identb", [P, P], BF16, kind="ExternalInput").ap()
    io["y"] = nc.dram_tensor(
        "y", [cfg.shard, cfg.D], F32, kind="ExternalOutput"
    ).ap()

    with tile.TileContext(nc) as tc:
        build(tc, io, cfg, meta)
    nc.compile()
    _CACHE[key] = nc
    return nc


def kernel(**inputs):
    from concourse import bass_utils

    cfg = Cfg()
    pre, meta = preprocess(inputs["edge_index"], cfg)
    nc = _build_program(cfg, meta)
    in_maps = make_in_maps(inputs, pre, cfg, meta)
    res = bass_utils.run_bass_kernel_spmd(nc, in_maps, core_ids=list(range(cfg.M)))
    order, M = meta["order"], cfg.M
    out = np.zeros((cfg.N, cfg.D), np.float32)
    for c in range(M):
        nodes = order[np.arange(cfg.shard) * M + c]
        out[nodes] = res.results[c]["y"]
    return out.astype(np.float32)


# revision 31
# speedup vs baseline: 1.1573x; 1.1573x over previous
"""GATv2 (3-layer, 4-head) message-passing kernel for Trainium2, 8-core SPMD.

V5 design: degree-bucketed destination-aligned edge layout.

Nodes are sorted by (in-)degree and dealt round-robin to the 8 cores, so each
core's chunk t of 128 destination slots covers the global degree-rank band
[1024t, 1024t+1024) and every core shares one per-chunk edge depth J[t]
(SPMD: one program).  Edge slot (p, j) of a chunk holds the j-th incoming
edge of destination slot p — so xr[dst] is a per-partition broadcast, the
segment softmax denominator and the weighted feature sum are per-partition
reductions over j (done on the PE via identity-matmul accumulation), and the
V4 selection-matrix machinery (is_equal S build + dstl) disappears entirely.

Gathers (int16 SWDGE indices cap tables at 32768 rows):
  The AllGather table has 8*(6250+1) = 50008 rows; each core appends one
  ZERO row after its shard, giving known-zero rows in both halves.
  Per chunk, hi-table edges are placed FIRST (j < nhi_p), so
    gather_hi: table rows [32768, 50008), covers j < Jhi[t]  (pads -> zero row)
    gather_lo: table rows [0, 32768),     covers j < J[t]    (pads -> zero row)
  and m = (g_lo[:, :Jhi] += g_hi) + xr broadcast needs no per-slot indexing.

Per-edge math (the V4 algebraic trick, per-partition denominators):
  m = xl[src] + xr[dst];  lk = prelu(m, 0.2) * attJ;  e = sum_c lk
  ee = exp(e) * mask;  po = sum_j [m*ee | ee]  (PE identity accumulation)
  out = po[:,0:D]/po[:,D:] - xr + (bl + bias); then residual + LayerNorm.

ACT stays inside ONE function table set (natural_log_exp_and_others:
copy/prelu/exp/ln/relu), with rstd = exp(-0.5*ln(var+eps)) — no per-chunk
ACT_TABLE_LOAD switches (V4 paid 2x ~2.6us per chunk for Exp<->Sqrt).
"""

import os
import sys

sys.path.insert(0, "/opt/trn_rl_repo")

import ml_dtypes
import numpy as np

import concourse.bass as bass
import concourse.bacc as bacc
import concourse.tile as tile
from concourse import mybir

F32 = mybir.dt.float32
I32 = mybir.dt.int32
I16 = mybir.dt.int16
BF16 = mybir.dt.bfloat16
AF = mybir.ActivationFunctionType
ALU = mybir.AluOpType
AX = mybir.AxisListType

P = 128
NEG_SLOPE = 0.2
LN_EPS = 1e-5
DENOM_EPS = 1e-30
TBL_SPLIT = 32768

DBG_LAYERS = int(os.environ.get("GAT_LAYERS", "0"))  # 0 = all


class Cfg:
    def __init__(self, N=50000, D=128, H=4, L=3, n_cores=8):
        self.N, self.D, self.H, self.L, self.M = N, D, H, L, n_cores
        self.C = D // H
        assert N % n_cores == 0
        self.shard = N // n_cores  # 6250
        self.chunks = (self.shard + P - 1) // P  # 49
        self.tshard = self.shard + 1  # shard + zero row
        self.trows = self.tshard * n_cores  # 50008


# ----------------------------------------------------------------------------
# Host preprocessing
# ----------------------------------------------------------------------------

def _wrap16(a):
    """Linear idx list [n] -> dma_gather layout [128, n/16] int16 (value for
    gathered row i sits at partition i%16, col i//16; replicated across the 8
    Q7 groups)."""
    n = len(a)
    assert n % 16 == 0
    a16 = a.reshape(-1, 16).T.astype(np.int16)
    return np.ascontiguousarray(np.tile(a16, (8, 1)))


def preprocess(edge_index, cfg):
    N, M, shard, chunks = cfg.N, cfg.M, cfg.shard, cfg.chunks
    ei = np.asarray(edge_index)
    loops = np.arange(N, dtype=np.int64)
    src = np.concatenate([ei[0].astype(np.int64), loops])
    dst = np.concatenate([ei[1].astype(np.int64), loops])

    deg = np.bincount(dst, minlength=N)

    # One fixpoint step: a degree-only sort gives provisional table rows and
    # hence provisional per-dst lo/hi counts, which become the secondary sort
    # key; the FINAL rows (and true nlo/nhi used for placement) come from the
    # final (deg desc, nlo desc) order. Chunks then have tight nlo AND nhi
    # ranges -> compact disjoint lo/hi gather regions.
    order0 = np.argsort(-deg, kind="stable")
    rank0 = np.empty(N, np.int64)
    rank0[order0] = np.arange(N)
    row0 = (rank0 % M) * cfg.tshard + rank0 // M
    hi0 = row0[src] >= TBL_SPLIT
    nlo0 = deg - np.bincount(
        dst, weights=hi0.astype(np.float64), minlength=N
    ).astype(np.int64)

    order = np.lexsort((-nlo0, -deg))  # rank -> node
    rank_of = np.empty(N, np.int64)
    rank_of[order] = np.arange(N)
    core_of = rank_of % M
    slot_of = rank_of // M
    table_row = core_of * cfg.tshard + slot_of  # FINAL AllGather-table rows
    ishi_e = table_row[src] >= TBL_SPLIT
    nhi = np.bincount(dst, weights=ishi_e.astype(np.float64), minlength=N)
    nhi = nhi.astype(np.int64)
    nlo = deg - nhi

    # zero rows: core c's row c*tshard + shard
    ZLO = 0 * cfg.tshard + shard  # 6250 (< 32768)
    ZHI = (M - 1) * cfg.tshard + shard - TBL_SPLIT  # 50007 - 32768 = 17239
    assert ZLO < TBL_SPLIT and 0 <= ZHI < 32768

    # per-chunk shared depths from global rank bands
    band = P * M
    Jlo = np.zeros(chunks, np.int64)
    Jhi = np.zeros(chunks, np.int64)
    for t in range(chunks):
        b = order[t * band : min(t * band + band, N)]
        Jlo[t] = nlo[b].max()
        Jhi[t] = nhi[b].max()
    Jlo = np.maximum(Jlo, 1)
    Jhi = np.maximum(Jhi, 1)
    J = Jlo + Jhi

    # per-edge placement: lo edges at j in [0, nlo_p), hi at [Jlo, Jlo+nhi_p)
    core_e = core_of[dst]
    slot_e = slot_of[dst]
    srow_e = table_row[src]
    oe = np.lexsort((np.arange(len(src)), ishi_e, slot_e, core_e))
    core_s, slot_s, srow_s, ishi_s = core_e[oe], slot_e[oe], srow_e[oe], ishi_e[oe]
    gid = core_s * shard + slot_s
    newgrp = np.r_[True, gid[1:] != gid[:-1]]
    idx = np.arange(len(gid))
    grp_start = np.maximum.accumulate(np.where(newgrp, idx, 0))
    jpos = idx - grp_start  # j within dst (lo edges first, 0-based)
    # hi edges' local index within the hi block: jpos - nlo[dst]
    dst_node_s = dst[oe]
    jhi_pos = jpos - nlo[dst_node_s]

    ch_s = slot_s // P
    p_s = slot_s % P

    pre = []
    for c in range(M):
        selc = core_s == c
        idx_lo, idx_hi = [], []
        for t in range(chunks):
            Jlt, Jht = int(Jlo[t]), int(Jhi[t])
            sel = selc & (ch_s == t)
            sj, sp, sr, sh = jpos[sel], p_s[sel], srow_s[sel], ishi_s[sel]
            sjh = jhi_pos[sel]
            A = np.full(Jlt * P, ZLO, dtype=np.int64)
            lo = ~sh
            A[sj[lo] * P + sp[lo]] = sr[lo]
            B = np.full(Jht * P, ZHI, dtype=np.int64)
            B[sjh[sh] * P + sp[sh]] = sr[sh] - TBL_SPLIT
            assert A.max() < TBL_SPLIT and B.max() < TBL_SPLIT
            idx_lo.append(_wrap16(A))
            idx_hi.append(_wrap16(B))
        pre.append({"idx_lo": idx_lo, "idx_hi": idx_hi})

    meta = {
        "J": tuple(int(x) for x in J),
        "Jlo": tuple(int(x) for x in Jlo),
        "Jhi": tuple(int(x) for x in Jhi),
        "order": order,
        "core_of": core_of,
        "slot_of": slot_of,
    }
    return pre, meta


# ----------------------------------------------------------------------------
# Kernel builder
# ----------------------------------------------------------------------------

def build(tc, io, cfg, meta):
    from contextlib import ExitStack

    nc = tc.nc
    D, H, L, C = cfg.D, cfg.H, cfg.L, cfg.C
    shard, chunks = cfg.shard, cfg.chunks
    J, Jhi = meta["J"], meta["Jhi"]
    Jmax = max(J)

    Jlo = meta["Jlo"]
    olo = np.concatenate([[0], np.cumsum(np.array(Jlo) * 8)])
    ohi = np.concatenate([[0], np.cumsum(np.array(Jhi) * 8)])

    ctx = ExitStack()
    dram = ctx.enter_context(tc.tile_pool(name="drampool", bufs=1, space="DRAM"))
    consts = ctx.enter_context(tc.tile_pool(name="consts", bufs=1))
    lconsts = ctx.enter_context(tc.tile_pool(name="lconsts", bufs=1))
    xtp = ctx.enter_context(tc.tile_pool(name="xtp", bufs=1))
    nodep = ctx.enter_context(tc.tile_pool(name="nodep", bufs=3))
    edgep = ctx.enter_context(tc.tile_pool(name="edgep", bufs=4))
    smallp = ctx.enter_context(tc.tile_pool(name="smallp", bufs=5))
    ps_o = ctx.enter_context(tc.tile_pool(name="ps_o", bufs=2, space="PSUM"))
    ps_n = ctx.enter_context(tc.tile_pool(name="ps_n", bufs=2, space="PSUM"))
    ps_t = ctx.enter_context(tc.tile_pool(name="ps_t", bufs=2, space="PSUM"))

    # internal DRAM
    xl_sh = [dram.tile([cfg.tshard, D], BF16, name=f"xl_sh{l}") for l in range(L)]
    xl_all = [
        dram.tile([cfg.trows, D], BF16, name=f"xl_all{l}", addr_space="Shared")
        for l in range(L)
    ]

    # constants resident in SBUF
    ident_sb = consts.tile([P, P], F32, name="ident_sb")
    nc.sync.dma_start(out=ident_sb[:], in_=io["ident"][:, :])
    identb_sb = consts.tile([P, P], BF16, name="identb_sb")
    nc.sync.dma_start(out=identb_sb[:], in_=io["identb"][:, :])
    zrow_sb = consts.tile([P, D], BF16, name="zrow_sb")
    nc.vector.memset(zrow_sb[:], 0.0)
    eps_sb = consts.tile([P, 1], F32, name="eps_sb")
    nc.vector.memset(eps_sb[:], LN_EPS)

    # Pin the ACT table to natural_log_exp_and_others (set 6: copy, prelu,
    # exp, ln, relu) — one load for the whole program, no per-chunk switches.
    nc.scalar.add_instruction(
        mybir.InstLoadActFuncSet(act_func_set_id=6, name="act_set6_load")
    )

    # layer-invariant per-chunk tables, SBUF-resident
    idxlo_sb = consts.tile([P, int(olo[-1])], I16, name="idxlo_sb")
    nc.sync.dma_start(out=idxlo_sb[:], in_=io["idxlo_all"][:, :])
    idxhi_sb = consts.tile([P, int(ohi[-1])], I16, name="idxhi_sb")
    nc.sync.dma_start(out=idxhi_sb[:], in_=io["idxhi_all"][:, :])

    # x transposed [D, shard] + residual input [node, D], SBUF-resident.
    # Ping-pong by layer so cross-chunk subtile writes never chain onto the
    # previous layer's reads (whole-tile dependency serialization).
    xT2 = [xtp.tile([P, chunks * P], BF16, name=f"xT_sb{i}") for i in range(2)]
    xq2 = [xtp.tile([P, chunks * D], BF16, name=f"xq_sb{i}") for i in range(2)]
    xr_sb = xtp.tile([P, chunks * D], BF16, name="xr_sb")

    # prologue: transpose x_shard into xT2[0]; keep x in xq2[0]
    for t in range(chunks):
        nt = min(P, shard - t * P)
        xq0 = nodep.tile([P, D], F32, name="xq0")
        nc.sync.dma_start(out=xq0[:nt, :], in_=io["x_shard"][t * P : t * P + nt, :])
        nc.scalar.activation(
            out=xq2[0][:nt, t * D : (t + 1) * D], in_=xq0[:nt, :], func=AF.Copy
        )
        psT = ps_t.tile([P, 512], F32, name="psT", tag="psT")
        nc.tensor.transpose(
            out=psT[:, :nt], in_=xq0[:nt, :], identity=ident_sb[:nt, :nt]
        )
        nc.scalar.activation(
            out=xT2[0][:, t * P : t * P + nt], in_=psT[:, :nt], func=AF.Copy
        )

    L_eff = DBG_LAYERS if DBG_LAYERS else L
    for l in range(L_eff):
        xT_sb = xT2[l % 2]
        xq_sb = xq2[l % 2]
        xT_nx = xT2[(l + 1) % 2]
        xq_nx = xq2[(l + 1) % 2]
        # per-layer constants
        wl_sb = lconsts.tile([P, D], BF16, name="wl_sb")
        nc.sync.dma_start(out=wl_sb[:], in_=io["Wl16"][l, :, :])
        wr_sb = lconsts.tile([P, D], BF16, name="wr_sb")
        nc.sync.dma_start(out=wr_sb[:], in_=io["Wr16"][l, :, :])
        attJ_sb = lconsts.tile([P, Jmax * D], BF16, name="attJ_sb")
        nc.gpsimd.dma_start(
            out=attJ_sb[:], in_=_row_bcast(io["attJ16"], l, P, Jmax * D)
        )
        bc_sb = lconsts.tile([P, D], F32, name="bc_sb")
        nc.gpsimd.dma_start(out=bc_sb[:], in_=_row_bcast(io["bc"], l, P, D))
        cvec_sb = lconsts.tile([P, D], F32, name="cvec_sb")
        nc.gpsimd.dma_start(out=cvec_sb[:], in_=_row_bcast(io["cvec"], l, P, D))
        gamma_sb = lconsts.tile([P, D], F32, name="gamma_sb")
        nc.gpsimd.dma_start(out=gamma_sb[:], in_=_row_bcast(io["gamma"], l, P, D))
        beta_sb = lconsts.tile([P, D], F32, name="beta_sb")
        nc.gpsimd.dma_start(out=beta_sb[:], in_=_row_bcast(io["beta"], l, P, D))

        # --------------------------------------------------------------
        # node phase: xl = x@Wl -> xl_sh (bf16); xr = x@Wr + (bl+br) -> xr_sb
        # --------------------------------------------------------------
        for t in range(chunks):
            nt = min(P, shard - t * P)
            lhsT = xT_sb[:, t * P : t * P + nt]
            ps_xl = ps_n.tile([P, 512], F32, name="ps_xl", tag="ps_n")
            nc.tensor.matmul(
                out=ps_xl[:nt, 0:D], lhsT=lhsT, rhs=wl_sb[:], start=True, stop=True
            )
            xl_o = nodep.tile([P, D], BF16, name="xl_o")
            nc.scalar.activation(out=xl_o[:nt, :], in_=ps_xl[:nt, 0:D], func=AF.Copy)
            nc.sync.dma_start(out=xl_sh[l][t * P : t * P + nt, :], in_=xl_o[:nt, :])

            ps_xr = ps_n.tile([P, 512], F32, name="ps_xr", tag="ps_n")
            nc.tensor.matmul(
                out=ps_xr[:nt, 0:D], lhsT=lhsT, rhs=wr_sb[:], start=True, stop=True
            )
            nc.vector.tensor_tensor(
                out=xr_sb[:nt, t * D : (t + 1) * D], in0=ps_xr[:nt, 0:D],
                in1=bc_sb[:nt, :], op=ALU.add,
            )
        # pad row = -1e4*sign(att): pad-slot logits underflow exp() to 0,
        # so no per-edge validity mask is needed.
        pad_sb = nodep.tile([P, D], BF16, name="pad_sb")
        nc.sync.dma_start(out=pad_sb[0:1, :], in_=io["sgn16"][l, :, :])
        nc.sync.dma_start(out=xl_sh[l][shard : shard + 1, :], in_=pad_sb[0:1, :])

        # --------------------------------------------------------------
        # AllGather xl across the 8 cores
        # --------------------------------------------------------------
        nc.gpsimd.collective_compute(
            "AllGather",
            ALU.bypass,
            replica_groups=[list(range(cfg.M))],
            ins=[xl_sh[l][:, :].opt()],
            outs=[xl_all[l][:, :].opt()],
        )

        # --------------------------------------------------------------
        # edge phase
        # --------------------------------------------------------------
        for ch in range(chunks):
            Jt, Jlt, Jht = J[ch], Jlo[ch], Jhi[ch]
            nt = min(P, shard - ch * P)
            rows = slice(ch * P, ch * P + nt)
            xr_ch = xr_sb[:, ch * D : (ch + 1) * D]

            # four gathers (one per SWDGE queue / Q7 core pair, concurrent)
            # fill disjoint j-ranges of one tile; pads hit shard-end zero rows
            g = edgep.tile([P, Jt, D], BF16, name="g")
            la = (Jlt + 1) // 2
            ha = (Jht + 1) // 2
            parts = [
                (0, la, idxlo_sb, olo[ch] * 1, 0, 0),
                (la, Jlt, idxlo_sb, olo[ch] + la * 8, 0, 1),
                (Jlt, Jlt + ha, idxhi_sb, ohi[ch] * 1, 1, 2),
                (Jlt + ha, Jt, idxhi_sb, ohi[ch] + ha * 8, 1, 3),
            ]
            for j0, j1, itile, ioff, ishi, q in parts:
                nj = j1 - j0
                if nj <= 0:
                    continue
                tbl = (
                    xl_all[l][TBL_SPLIT : cfg.trows, :]
                    if ishi
                    else xl_all[l][0:TBL_SPLIT, :]
                )
                nc.gpsimd.dma_gather(
                    out_ap=g[:, j0:j1, :],
                    in_ap=tbl,
                    idxs_ap=itile[:, int(ioff) : int(ioff) + nj * 8],
                    num_idxs=nj * P,
                    num_idxs_reg=nj * P,
                    elem_size=D,
                    single_packet=False,
                    queue_num=q,
                )

            # m = xl[src] + xr[dst]  (in place: g becomes m)
            nc.vector.tensor_tensor(
                out=g[:, :, :],
                in0=g[:, :, :],
                in1=xr_ch.unsqueeze(1).to_broadcast([P, Jt, D]),
                op=ALU.add,
            )

            # lk = prelu(m) * att (att pre-replicated along J)
            lk = edgep.tile([P, Jt, D], BF16, name="lk")
            nc.scalar.activation(
                out=lk[:, :, :], in_=g[:, :, :], func=AF.Prelu, alpha=NEG_SLOPE
            )
            nc.vector.tensor_tensor(
                out=lk[:, :, :],
                in0=lk[:, :, :],
                in1=attJ_sb[:, 0 : Jt * D].rearrange("p (j d) -> p j d", j=Jt),
                op=ALU.mult,
            )
            lg = smallp.tile([P, Jt, H], F32, name="lg")
            nc.vector.reduce_sum(
                out=lg[:, :, :],
                in_=lk[:, :, :].rearrange("p j (h c) -> p j h c", h=H),
                axis=AX.X,
            )

            # ee = exp(e) * mask (small); denominators on the DVE (tiny
            # strided reduce); ee broadcast over head cols via ACT into lk
            # (dead after lg), then lk *= m  ->  lk = ee * m
            eem = smallp.tile([P, Jt, H], BF16, name="eem")
            nc.scalar.activation(out=eem[:, :, :], in_=lg[:, :, :], func=AF.Exp)
            dns = smallp.tile([P, H], F32, name="dns")
            nc.vector.reduce_sum(
                out=dns[:, :],
                in_=eem[:, :, :].rearrange("p j h -> p h j"),
                axis=AX.X,
            )
            nc.scalar.activation(
                out=lk[:, :, :].rearrange("p j (h c) -> p j h c", h=H),
                in_=eem[:, :, :].unsqueeze(3).to_broadcast([P, Jt, H, C]),
                func=AF.Copy,
            )
            nc.vector.tensor_tensor(
                out=lk[:, :, :], in0=lk[:, :, :], in1=g[:, :, :], op=ALU.mult
            )

            # per-dst weighted sums over j on the PE: po = sum_j ee*m
            po_b = ps_o.tile([P, 512], F32, name="po")
            po = po_b[:, 0:D]
            for j in range(Jt):
                nc.tensor.matmul(
                    out=po[:, :],
                    lhsT=identb_sb[:, :],
                    rhs=lk[:, j, :],
                    start=(j == 0),
                    stop=(j == Jt - 1),
                )

            dn = smallp.tile([P, H], F32, name="dn")
            nc.vector.tensor_scalar(
                out=dn[:, :], in0=dns[:, :], scalar1=DENOM_EPS,
                scalar2=None, op0=ALU.add,
            )
            rd = smallp.tile([P, H], F32, name="rd")
            nc.vector.reciprocal(out=rd[:, :], in_=dn[:, :])

            onrm = smallp.tile([P, D], F32, name="onrm")
            nc.vector.tensor_tensor(
                out=onrm[:, :].rearrange("p (h c) -> p h c", h=H),
                in0=po[:, :].rearrange("p (h c) -> p h c", h=H),
                in1=rd[:, :].unsqueeze(2).to_broadcast([P, H, C]),
                op=ALU.mult,
            )

            # h = onrm - xr[dst] + (bl + gat_bias); then residual + LN
            # (in-place chain on the onrm tile)
            t3 = onrm
            nc.vector.tensor_tensor(
                out=t3[:nt, :], in0=onrm[:nt, :], in1=xr_ch[:nt, :],
                op=ALU.subtract,
            )
            nc.vector.tensor_tensor(
                out=t3[:nt, :], in0=t3[:nt, :], in1=cvec_sb[:nt, :], op=ALU.add
            )
            nc.vector.tensor_tensor(
                out=t3[:nt, :], in0=t3[:nt, :],
                in1=xq_sb[:nt, ch * D : (ch + 1) * D], op=ALU.add,
            )

            st6 = smallp.tile([P, 6], F32, name="st6")
            nc.vector.bn_stats(out=st6[:nt, :], in_=t3[:nt, :])
            mv = smallp.tile([P, 2], F32, name="mv")
            nc.vector.bn_aggr(out=mv[:nt, :], in_=st6[:nt, :])
            # rstd = exp(-0.5 * ln(var + eps)) — both funcs live in set 6
            lnv = smallp.tile([P, 1], F32, name="lnv")
            nc.scalar.activation(
                out=lnv[:nt, :], in_=mv[:nt, 1:2], func=AF.Ln, bias=eps_sb[:nt, :]
            )
            rstd = smallp.tile([P, 1], F32, name="rstd")
            nc.scalar.activation(
                out=rstd[:nt, :], in_=lnv[:nt, :], func=AF.Exp, scale=-0.5
            )

            y3 = smallp.tile([P, D], F32, name="y3")
            nc.vector.tensor_scalar(
                out=y3[:nt, :], in0=t3[:nt, :], scalar1=mv[:nt, 0:1],
                scalar2=rstd[:nt, :], op0=ALU.subtract, op1=ALU.mult,
            )
            nc.vector.tensor_tensor(
                out=y3[:nt, :], in0=y3[:nt, :], in1=gamma_sb[:nt, :], op=ALU.mult
            )
            nc.vector.tensor_tensor(
                out=y3[:nt, :], in0=y3[:nt, :], in1=beta_sb[:nt, :], op=ALU.add
            )

            if l < L_eff - 1:
                # relu -> next layer's residual input (SBUF) + transpose
                nc.scalar.activation(
                    out=xq_nx[:nt, ch * D : (ch + 1) * D], in_=y3[:nt, :],
                    func=AF.Relu,
                )
                psT2 = ps_t.tile([P, 1024], BF16, name="psT2", tag="psT")
                nc.tensor.transpose(
                    out=psT2[:, :nt], in_=xq_nx[:nt, ch * D : (ch + 1) * D],
                    identity=identb_sb[:nt, :nt],
                )
                nc.scalar.activation(
                    out=xT_nx[:, ch * P : ch * P + nt], in_=psT2[:, :nt],
                    func=AF.Copy,
                )
            else:
                nc.sync.dma_start(out=io["y"][rows, :], in_=y3[:nt, :])

    ctx.close()


def _row_bcast(ap, row, parts, d):
    """AP reading row `row` of a [R, 1, D] or [R, D] DRAM tensor, replicated
    across `parts` partitions (partition step 0)."""
    flat = ap[row] if ap.ndim == 3 else ap[row : row + 1]
    base = flat.opt()
    return bass.AP(tensor=base.tensor, offset=row * d, ap=[[0, parts], [1, d]])


# ----------------------------------------------------------------------------
# host-side inputs
# ----------------------------------------------------------------------------

def make_host_inputs(inputs, cfg, meta):
    L, D, H, C = cfg.L, cfg.D, cfg.H, cfg.C
    Jmax = max(meta["J"])
    Wl = np.asarray(inputs["Wl"], np.float32)
    bl = np.asarray(inputs["bl"], np.float32)
    br = np.asarray(inputs["br"], np.float32)
    att = np.asarray(inputs["att"], np.float32)
    gat_bias = np.asarray(inputs["bias"], np.float32)
    gamma = np.asarray(inputs["gamma"], np.float32)
    beta = np.asarray(inputs["beta"], np.float32)
    attJ = np.tile(att.reshape(L, 1, H * C), (1, Jmax, 1)).reshape(L, 1, Jmax * D)
    return {
        "Wl16": Wl.astype(ml_dtypes.bfloat16),
        "Wr16": np.asarray(inputs["Wr"], np.float32).astype(ml_dtypes.bfloat16),
        "attJ16": attJ.astype(ml_dtypes.bfloat16),
        "bc": (bl + br).reshape(L, 1, D),
        "cvec": (bl + gat_bias).reshape(L, 1, D),
        "gamma": gamma.reshape(L, 1, D),
        "beta": beta.reshape(L, 1, D),
        "sgn16": (-1e4 * np.sign(att).reshape(L, 1, D)).astype(
            ml_dtypes.bfloat16
        ),
        "ident": np.eye(P, dtype=np.float32),
        "identb": np.eye(P, dtype=np.float32).astype(ml_dtypes.bfloat16),
    }


def make_in_maps(inputs, pre, cfg, meta):
    x = np.asarray(inputs["fine_poi_x"], np.float32)
    shared = make_host_inputs(inputs, cfg, meta)
    order, core_of, slot_of = meta["order"], meta["core_of"], meta["slot_of"]
    in_maps = []
    for c in range(cfg.M):
        m = dict(shared)
        # x rows of core c in slot order: node at (c, slot s) = order[s*M + c]
        nodes = order[np.arange(cfg.shard) * cfg.M + c]
        m["x_shard"] = np.ascontiguousarray(x[nodes])
        m["idxlo_all"] = np.concatenate(pre[c]["idx_lo"], axis=1)
        m["idxhi_all"] = np.concatenate(pre[c]["idx_hi"], axis=1)
        in_maps.append(m)
    return in_maps


# ----------------------------------------------------------------------------
# program assembly + execution
# ----------------------------------------------------------------------------

_CACHE = {}


def _build_program(cfg, meta):
    key = (cfg.N, cfg.D, cfg.H, cfg.L, cfg.M, meta["Jlo"], meta["Jhi"])
    if key in _CACHE:
        return _CACHE[key]
    nc = bacc.Bacc(
        "TRN2", target_bir_lowering=False, debug=False, num_devices=cfg.M,
        num_swdge_queues=4,
    )
    J, Jhi = meta["J"], meta["Jhi"]
    Jmax = max(J)
    io = {}
    io["x_shard"] = nc.dram_tensor(
        "x_shard", [cfg.shard, cfg.D], F32, kind="ExternalInput"
    ).ap()
    Jlo = meta["Jlo"]
    io["idxlo_all"] = nc.dram_tensor(
        "idxlo_all", [P, sum(Jlo) * 8], I16, kind="ExternalInput"
    ).ap()
    io["idxhi_all"] = nc.dram_tensor(
        "idxhi_all", [P, sum(Jhi) * 8], I16, kind="ExternalInput"
    ).ap()
    io["sgn16"] = nc.dram_tensor(
        "sgn16", [cfg.L, 1, cfg.D], BF16, kind="ExternalInput"
    ).ap()
    io["Wl16"] = nc.dram_tensor(
        "Wl16", [cfg.L, cfg.D, cfg.D], BF16, kind="ExternalInput"
    ).ap()
    io["Wr16"] = nc.dram_tensor(
        "Wr16", [cfg.L, cfg.D, cfg.D], BF16, kind="ExternalInput"
    ).ap()
    io["attJ16"] = nc.dram_tensor(
        "attJ16", [cfg.L, 1, Jmax * cfg.D], BF16, kind="ExternalInput"
    ).ap()
    for nm in ["bc", "cvec", "gamma", "beta"]:
        io[nm] = nc.dram_tensor(
            nm, [cfg.L, 1, cfg.D], F32, kind="ExternalInput"
        ).ap()
    io["ident"] = nc.dram_tensor("ident", [P, P], F32, kind="ExternalInput").ap()
    io["identb"] = nc.dram_tensor("identb", [P, P], BF16, kind="ExternalInput").ap()
    io["y"] = nc.dram_tensor(
        "y", [cfg.shard, cfg.D], F32, kind="ExternalOutput"
    ).ap()

    with tile.TileContext(nc) as tc:
        build(tc, io, cfg, meta)
    nc.compile()
    _CACHE[key] = nc
    return nc


def kernel(**inputs):
    from concourse import bass_utils

    cfg = Cfg()
    pre, meta = preprocess(inputs["edge_index"], cfg)
    nc = _build_program(cfg, meta)
    in_maps = make_in_maps(inputs, pre, cfg, meta)
    res = bass_utils.run_bass_kernel_spmd(nc, in_maps, core_ids=list(range(cfg.M)))
    order, M = meta["order"], cfg.M
    out = np.zeros((cfg.N, cfg.D), np.float32)
    for c in range(M):
        nodes = order[np.arange(cfg.shard) * M + c]
        out[nodes] = res.results[c]["y"]
    return out.astype(np.float32)
